# revision 45
# baseline (speedup 1.0000x reference)
"""BarrierNet Trainium2 kernel: 8-core data-parallel Bass/Tile implementation.

Takes full inputs, shards batch across 8 NeuronCores, returns full output.

Per-core structure (S = 131072 samples, 16 spans of 8192):
  - obs loaded naturally as f32r: partition p of a span holds samples
    [base+64p, base+64p+64) (4KB contiguous per partition -> full DMA
    efficiency); next span's load/extract/transpose pipelined one span ahead.
  - T1: PE block-transposes [128,128] blocks into packed obsT (rows
    16*(t%8)+f), all-f32r (moving identity f32r: 1.5 cyc/row).
  - MLP with all matmul moving operands at full streaming rate:
      L1: K=32 zero-padded w1 strips (f32r, 1 cyc/row), 2 concurrent
          row-tiles per [128,1024] PSUM tile (bufs=2 -> ACT/PE overlap);
          silu via ACT -> y1sT bf16.
      L2: col-tiled K=128 (w2 bf16, 1 cyc/row); silu via ACT -> y2sT bf16.
      L3: col-tiled block-diag w3 (bf16) -> dense natp[32*hs+2j+ch, c4*128+p]
          (M=32 with zero cols so the whole tile is written).
  - Transpose-back: natp -> unT (bf16) -> 4 dense [128,128] bf16 PE
    transposes -> u_nom copied (sample,channel)-interleaved into uxy planes.
  - Barrier math (dCVaR-CBF + closed-form QP) in fp32 natural layout, split
    across DVE and GpSimd; only the largest-sigma GMM mode is evaluated
    (means equal, sigma monotone in variance, CVaR coeff > 0).
    sqrt via magic-seed rsqrt + 1 Newton step (rel err ~2e-3 << tolerance).
    Last group's barrier runs in per-span chunks to shorten the kernel tail.
  - Output assembled run-major: partition p holds samples 64p..64p+63
    interleaved (x,y) -> 512-byte contiguous runs per partition.

Engine budget per core (CoreSim cost model): ACT ~173us (binding: 10.2k
silu lane-cols/span + per-instr access latency), PE ~136us, DVE ~58us,
GpSimd ~15us, DMA fully hidden. Simulated total ~185us vs 532us for the
previous all-fp32 version.
"""
import sys

sys.path.insert(0, '/opt/trn_rl_repo')

from contextlib import ExitStack

import numpy as np
import ml_dtypes

import concourse.bass as bass  # noqa: F401
import concourse.tile as tile
from concourse import bacc, mybir
from concourse.bass_utils import run_bass_kernel_spmd

N_CORES = 8
B = 1_048_576
NF, H1, H2, NC = 16, 128, 32, 2
S = B // N_CORES              # samples per core
SAFE_DIST = 0.8
ALPHA = 2.0
CVAR_COEFF = 1.7549833193248685
SIG_MAX_VAR = 0.3 * 0.3       # largest GMM mode variance (worst-case mode)
EPS_SIG = 1e-8
EPS_DIV = 1e-12

TR = 64                        # samples per partition run
V = 128 * TR                   # natural span = 8192 samples
GRP = 4                        # spans per barrier group
FP32 = mybir.dt.float32
F32R = mybir.dt.float32r
BF16 = mybir.dt.bfloat16
I32 = mybir.dt.int32
BF = ml_dtypes.bfloat16

_cached = {}


def build(s_samples=S, n_devices=N_CORES, act_func=None):
    nc = bacc.Bacc("TRN2", target_bir_lowering=False, debug=False,
                   num_devices=n_devices)
    obs_ap = nc.dram_tensor("obs", [s_samples, NF], F32R, kind="ExternalInput").ap()
    # blobF (f32r): identf(128) | w1pad0(128) | w1pad1(128)
    # blobB (bf16): identb(128) | w2rep(128) | w3stack(32)
    # blobS (fp32): b1c(1) | b2rep(1) | b3c(2)
    bl1_ap = nc.dram_tensor("blobF", [128, 384], F32R, kind="ExternalInput").ap()
    bl2_ap = nc.dram_tensor("blobB", [128, 288], BF16, kind="ExternalInput").ap()
    bl3_ap = nc.dram_tensor("blobS", [128, 4], FP32, kind="ExternalInput").ap()
    out_ap = nc.dram_tensor("out", [s_samples, NC], FP32, kind="ExternalOutput").ap()

    with tile.TileContext(nc) as tc, ExitStack() as ctx:
        kernel_body(ctx, tc, out_ap, obs_ap, bl1_ap, bl2_ap, bl3_ap,
                    s_samples, act_func)
    nc.compile()
    return nc


def kernel_body(ctx, tc, out_ap, obs_ap, bl1_ap, bl2_ap, bl3_ap,
                s_samples, act_func=None):
    nc = tc.nc
    nspan = s_samples // V
    ngrp = nspan // GRP
    SILU = act_func or mybir.ActivationFunctionType.Silu
    ALU = mybir.AluOpType

    const = ctx.enter_context(tc.tile_pool(name="const", bufs=1))
    nat_pool = ctx.enter_context(tc.tile_pool(name="nat", bufs=2))
    obsT_pool = ctx.enter_context(tc.tile_pool(name="obsT", bufs=2))
    y1_pool = ctx.enter_context(tc.tile_pool(name="y1", bufs=2))
    y2_pool = ctx.enter_context(tc.tile_pool(name="y2", bufs=2))
    unT_pool = ctx.enter_context(tc.tile_pool(name="unT", bufs=2))
    plane_pool = ctx.enter_context(tc.tile_pool(name="plane", bufs=2))
    scr_pool = ctx.enter_context(tc.tile_pool(name="scr", bufs=1))
    outb_pool = ctx.enter_context(tc.tile_pool(name="outb", bufs=2))

    ps_l1 = ctx.enter_context(tc.tile_pool(name="ps_l1", bufs=2, space="PSUM"))
    ps_tp = ctx.enter_context(tc.tile_pool(name="ps_tp", bufs=1, space="PSUM"))
    ps_y2 = ctx.enter_context(tc.tile_pool(name="ps_y2", bufs=2, space="PSUM"))
    # natp and t2p alternate through one single-bank ring (sequential uses)
    ps_mis = ctx.enter_context(tc.tile_pool(name="ps_mis", bufs=1, space="PSUM"))

    # constants: packed blobs (one DMA per dtype class)
    blobF = const.tile([128, 384], F32R)
    blobB = const.tile([128, 288], BF16)
    blobS = const.tile([128, 4], FP32)
    nc.sync.dma_start(blobF[:], bl1_ap[:])
    identF = blobF[:, 0:128]
    w1pads = (blobF[:, 128:256], blobF[:, 256:384])
    identB = blobB[:, 0:128]
    w2rep = blobB[:, 128:256]
    w3stk = blobB[:, 256:288]
    b1c = blobS[:, 0:1]
    b2rep = blobS[:, 1:2]
    b3c = blobS[:, 2:4]

    V_ = nc.vector
    G_ = nc.gpsimd

    planes = {}

    def group_tiles(gi):
        if gi not in planes:
            planes[gi] = {
                nm: plane_pool.tile([128, GRP * TR * (2 if nm == "uxy" else 1)],
                                    FP32, tag=nm, name=nm)
                for nm in ("relx", "rely", "hvx", "hvy", "uxy")}
        return planes[gi]

    def emit_load(span, split=False):
        obs_nat = nat_pool.tile([128, TR * NF], F32R, tag="obs_nat")
        base = span * V
        src = obs_ap[base:base + V, :].rearrange("(p t) f -> p (t f)", p=128)
        if split:
            nc.sync.dma_start(obs_nat[:, 0:512], src[:, 0:512])
            nc.sync.dma_start(obs_nat[:, 512:1024], src[:, 512:1024])
        else:
            nc.sync.dma_start(obs_nat[:], src)
        return obs_nat

    def emit_t1(span, obs_nat):
        gi, sl = span // GRP, span % GRP
        t = group_tiles(gi)
        # barrier input extraction (GpSimd; SBUF->SBUF)
        ob3 = obs_nat[:].rearrange("p (t f) -> p t f", f=NF)
        pl_sl = slice(sl * TR, (sl + 1) * TR)
        G_.tensor_copy(t["relx"][:, pl_sl], ob3[:, :, 6])
        G_.tensor_copy(t["rely"][:, pl_sl], ob3[:, :, 7])
        G_.tensor_copy(t["hvx"][:, pl_sl], ob3[:, :, 8])
        G_.tensor_copy(t["hvy"][:, pl_sl], ob3[:, :, 9])
        # T1: PE transpose natural -> packed obsT (rows 16*(t%8)+f)
        obsT = obsT_pool.tile([128, 1024], F32R, tag="obsT")
        for half in range(2):
            tp = ps_tp.tile([128, 512], F32R, tag="tp")
            for ci in range(4):
                c = half * 4 + ci
                nc.tensor.matmul(
                    tp[:, ci * 128:(ci + 1) * 128],
                    obs_nat[:, c * 128:(c + 1) * 128],
                    identF, is_transpose=True)
            V_.tensor_copy(obsT[:, half * 512:(half + 1) * 512], tp[:])
        return obsT

    def emit_barrier(g, tg, nchunks=1):
        P_ = G_
        # ======== barrier math (natural layout, fp32, per group) ========
        # nchunks>1 splits the group column-wise so early chunks' chains can
        # run during later spans' MLP (used for the last group to cut tail).
        Wg = GRP * TR
        Wc = Wg // nchunks
        outb = outb_pool.tile([128, GRP * 2 * TR], FP32, tag="outb")
        for c in range(nchunks):
            ps = slice(c * Wc, (c + 1) * Wc)
            us = slice(c * 2 * Wc, (c + 1) * 2 * Wc)
            relx, rely = tg["relx"][:, ps], tg["rely"][:, ps]
            hvx, hvy = tg["hvx"][:, ps], tg["hvy"][:, ps]
            xv = tg["uxy"][:, us].rearrange("p (w c) -> p w c", c=2)
            ux, uy = xv[:, :, 0], xv[:, :, 1]
            ov = outb[:, us].rearrange("p (w c) -> p w c", c=2)

            a = scr_pool.tile([128, Wc], FP32, tag="a", name="a")
            b_ = scr_pool.tile([128, Wc], FP32, tag="b", name="b_")
            rnsq = scr_pool.tile([128, Wc], FP32, tag="rnsq", name="rnsq")
            rdm = scr_pool.tile([128, Wc], FP32, tag="rdm", name="rdm")
            sig = scr_pool.tile([128, Wc], FP32, tag="sig", name="sig")
            yv = scr_pool.tile([128, Wc], FP32, tag="yv", name="yv")
            k = scr_pool.tile([128, Wc], FP32, tag="k", name="k")
            w = scr_pool.tile([128, Wc], FP32, tag="w", name="w")

            # u_nom += b3 (in place; feeds the dot product and the output)
            V_.tensor_scalar(ux, ux, b3c[:, 0:1], None, ALU.add)
            V_.tensor_scalar(uy, uy, b3c[:, 1:2], None, ALU.add)
            # rnsq, rdm (GpSimd)
            P_.tensor_mul(a[:], relx, relx)
            P_.tensor_mul(b_[:], rely, rely)
            P_.tensor_add(rnsq[:], a[:], b_[:])
            P_.tensor_mul(a[:], hvx, relx)
            P_.tensor_mul(b_[:], hvy, rely)
            P_.tensor_add(rdm[:], a[:], b_[:])      # rel_dot_mu / 2
            # sigma = sqrt(x), x = 4*var*rnsq + eps_sig: magic rsqrt + 1 NR
            V_.tensor_scalar(sig[:], rnsq[:], 4.0 * SIG_MAX_VAR, EPS_SIG,
                             ALU.mult, ALU.add)
            V_.tensor_copy(a[:], sig[:].bitcast(I32))          # f = float(i)
            V_.tensor_scalar(a[:], a[:], -0.5, 1597463007.0, ALU.mult, ALU.add)
            V_.tensor_copy(yv[:].bitcast(I32), a[:])           # y0 bits
            P_.tensor_mul(a[:], yv[:], yv[:])
            P_.tensor_mul(a[:], a[:], sig[:])
            V_.tensor_scalar(a[:], a[:], -0.5, 1.5, ALU.mult, ALU.add)
            P_.tensor_mul(yv[:], yv[:], a[:])
            V_.tensor_mul(sig[:], sig[:], yv[:])               # sqrt = x*rsqrt
            # k = rdm - rel.u - rnsq + CV/2*sigma ; viol/2 = k + S^2
            P_.tensor_mul(a[:], relx, ux)
            P_.tensor_mul(b_[:], rely, uy)
            V_.tensor_add(a[:], a[:], b_[:])
            V_.tensor_sub(k[:], rdm[:], a[:])
            V_.tensor_sub(k[:], k[:], rnsq[:])
            V_.scalar_tensor_tensor(k[:], sig[:], 0.5 * CVAR_COEFF, k[:],
                                    ALU.mult, ALU.add)
            # v = max(viol/2, 0) ; w = gnsq/2 = 2*rnsq + eps/2
            V_.tensor_scalar(k[:], k[:], SAFE_DIST ** 2, 0.0, ALU.add, ALU.max)
            V_.tensor_scalar(w[:], rnsq[:], 2.0, 0.5 * EPS_DIV,
                             ALU.mult, ALU.add)
            V_.reciprocal(w[:], w[:])
            V_.tensor_mul(k[:], k[:], w[:])                    # coef
            # u_safe = u + 2*coef*rel
            P_.tensor_mul(a[:], k[:], relx)
            V_.scalar_tensor_tensor(ov[:, :, 0], a[:], 2.0, ux,
                                    ALU.mult, ALU.add)
            P_.tensor_mul(b_[:], k[:], rely)
            V_.scalar_tensor_tensor(ov[:, :, 1], b_[:], 2.0, uy,
                                    ALU.mult, ALU.add)

            # store run-major for the spans this chunk fully covers
            spans_per_chunk = GRP // nchunks
            for s3 in range(spans_per_chunk):
                s2 = c * spans_per_chunk + s3
                base = (g * GRP + s2) * V
                dst = out_ap[base:base + V, :].rearrange(
                    "(p t) c -> p (t c)", p=128)
                nc.sync.dma_start(dst, outb[:, s2 * 2 * TR:(s2 + 1) * 2 * TR])

    obs_nxt = emit_load(0, split=True)
    nc.sync.dma_start(blobS[:], bl3_ap[:])
    nc.sync.dma_start(blobB[:], bl2_ap[:])
    obsT_nxt = emit_t1(0, obs_nxt)

    for span in range(nspan):
        g, sl = span // GRP, span % GRP
        obsT = obsT_nxt
        tg = group_tiles(g)
        uxy = tg["uxy"]

        # layouts: obsT col = c*128 + p; row = 16*(t%8) + f; c = t//8.
        # y1sT col = (t8*2 + h)*512 + c4*128 + p   (h: half, c4 = c%4)
        # y2sT col = hs*512 + c4*128 + p, hs = 2h+sub, groups t8 = 4*sub+j
        y1sT = y1_pool.tile([128, 8192], BF16, tag="y1sT")
        y2sT = y2_pool.tile([128, 2048], BF16, tag="y2sT")

        # ---- L1: 2 concurrent row-tiles per [128,1024] psum tile ----
        for h in range(2):
            hs_cols = slice(h * 512, (h + 1) * 512)
            for par in range(2):
                for qh in range(2):
                    l1 = ps_l1.tile([128, 1024], FP32, tag="l1")
                    for qq in range(2):
                        q = 2 * qh + qq
                        nc.tensor.matmul(
                            l1[:, qq * 512:(qq + 1) * 512],
                            w1pads[par][32 * q:32 * q + 32, :],
                            obsT[32 * q:32 * q + 32, hs_cols],
                            start=True, stop=True,
                            tile_position=(32 * q, 0))
                    t80 = 4 * qh + par
                    dst = y1sT[:].rearrange("p (t8 h2 n) -> p t8 h2 n",
                                            t8=8, h2=2)[:, t80:t80 + 3:2, h]
                    srcv = l1[:].rearrange("p (q n) -> p q n", q=2)
                    nc.scalar.activation(dst, srcv, SILU,
                                         bias=b1c[:, 0:1], scale=1.0)

        # ---- pipeline next span's load + extracts + T1 ----
        if span + 1 < nspan:
            obs_nxt = emit_load(span + 1)
            obsT_nxt = emit_t1(span + 1, obs_nxt)

        # ---- L2: col-tiled K=128 ----
        for hs in range(4):
            h, sub = hs // 2, hs % 2
            y2p = ps_y2.tile([128, 512], FP32, tag="y2p")
            for j in range(4):
                t8 = 4 * sub + j
                nc.tensor.matmul(
                    y2p[32 * j:32 * j + 32, :],
                    w2rep[:, 32 * j:32 * j + 32],
                    y1sT[:, (t8 * 2 + h) * 512:(t8 * 2 + h + 1) * 512],
                    start=True, stop=True,
                    tile_position=(0, 32 * j))
            nc.scalar.activation(y2sT[:, hs * 512:(hs + 1) * 512],
                                 y2p[:], SILU, bias=b2rep[:, 0:1], scale=1.0)

        # ---- L3: col-tiled block-diag w3 -> dense natural-ish psum ----
        # natp row = 32*hs + 2*j + ch, col = c4*128 + p
        natp = ps_mis.tile([128, 512], FP32, tag="mis", name="natp")
        for hs in range(4):
            nc.tensor.matmul(
                natp[32 * hs:32 * hs + 32, :],
                w3stk[:],
                y2sT[:, hs * 512:(hs + 1) * 512],
                start=True, stop=True,
                tile_position=(0, 32 * hs))
        unTt = unT_pool.tile([128, 512], BF16, tag="unT")
        V_.tensor_copy(unTt[:], natp[:])

        # ---- T2: dense transpose-back [128,128] blocks ----
        t2p = ps_mis.tile([128, 512], BF16, tag="mis", name="t2p")
        for c4 in range(4):
            nc.tensor.matmul(
                t2p[:, c4 * 128:(c4 + 1) * 128],
                unTt[:, c4 * 128:(c4 + 1) * 128],
                identB[:], is_transpose=True)
        # t2p col = c4*128 + 64h+32sub+(2j+ch) ; sample t = 32h+8c4+4sub+j
        # uxy col = sl*128 + 2t+ch = sl*128 + 64h+16c4+8sub+(2j+ch)
        inv = t2p[:].rearrange("p (c4 h sub jc) -> p h c4 sub jc",
                               c4=4, h=2, sub=2)[:, :, :, :, 0:8]
        outv = uxy[:, sl * 128:(sl + 1) * 128].rearrange(
            "p (h c4 sub jc) -> p h c4 sub jc", h=2, c4=4, sub=2)
        V_.tensor_copy(outv, inv)

        if sl == GRP - 1:
            emit_barrier(g, tg, nchunks=GRP if g == ngrp - 1 else 1)
            del planes[g]


def prep_consts(w1, b1, w2, b2, w3, b3):
    w1pad0 = np.zeros((128, 128), np.float32)
    w1pad1 = np.zeros((128, 128), np.float32)
    w2rep = np.zeros((128, 128), BF)
    w3stack = np.zeros((128, 32), BF)
    for q in range(4):
        w1pad0[32 * q:32 * q + 16, :] = w1.T          # even t8 groups
        w1pad1[32 * q + 16:32 * q + 32, :] = w1.T     # odd t8 groups
    for j in range(4):
        w2rep[:, 32 * j:32 * j + 32] = w2.T.astype(BF)
        w3stack[32 * j:32 * j + 32, 2 * j:2 * j + 2] = w3.T.astype(BF)
    b3c = np.empty((128, 2), np.float32)
    b3c[:, 0] = b3[0]
    b3c[:, 1] = b3[1]
    blobF = np.concatenate([np.eye(128, dtype=np.float32), w1pad0, w1pad1],
                           axis=1)
    blobB = np.concatenate([np.eye(128, dtype=BF), w2rep, w3stack], axis=1)
    blobS = np.concatenate([
        np.asarray(b1, np.float32).reshape(128, 1),
        np.tile(np.asarray(b2, np.float32), 4).reshape(128, 1),
        b3c], axis=1)
    return dict(blobF=np.ascontiguousarray(blobF),
                blobB=np.ascontiguousarray(blobB),
                blobS=np.ascontiguousarray(blobS))


def kernel(obs, w1, b1, w2, b2, w3, b3):
    obs = np.asarray(obs, np.float32)
    consts = prep_consts(np.asarray(w1, np.float32), np.asarray(b1, np.float32),
                         np.asarray(w2, np.float32), np.asarray(b2, np.float32),
                         np.asarray(w3, np.float32), np.asarray(b3, np.float32))
    if "nc" not in _cached:
        _cached["nc"] = build()
    nc = _cached["nc"]
    in_maps = []
    for c in range(N_CORES):
        m = {"obs": np.ascontiguousarray(obs[c * S:(c + 1) * S])}
        m.update(consts)
        in_maps.append(m)
    res = run_bass_kernel_spmd(nc, in_maps, list(range(N_CORES)))
    out = np.empty((B, NC), np.float32)
    for c in range(N_CORES):
        out[c * S:(c + 1) * S] = res.results[c]["out"]
    return out


# revision 51
# speedup vs baseline: 1.4024x; 1.4024x over previous
"""BarrierNet Trainium2 kernel: 8-core data-parallel Bass/Tile implementation.

Takes full inputs, shards batch across 8 NeuronCores, returns full output.

Per-core structure (S = 131072 samples, 16 spans of 8192):
  - obs loaded naturally as f32r: partition p of a span holds samples
    [base+64p, base+64p+64) (4KB contiguous per partition -> full DMA
    efficiency); next span's load/extract/transpose pipelined one span ahead.
  - T1: PE block-transposes [128,128] blocks into packed obsT (rows
    16*(t%8)+f), all-f32r (moving identity f32r: 1.5 cyc/row).
  - MLP with all matmul moving operands at full streaming rate:
      L1: K=32 zero-padded w1 strips (f32r, 1 cyc/row), 2 concurrent
          row-tiles per [128,1024] PSUM tile (bufs=2 -> ACT/PE overlap);
          silu via ACT -> y1sT bf16.
      L2: col-tiled K=128 (w2 bf16, 1 cyc/row); hs groups 0-2 drain through
          one [128,1536] ACT silu, group 3 via the shared psum ring -> y2sT
          bf16.
      L3: col-tiled block-diag w3 (bf16) -> dense natp[32*hs+2j+ch, c4*128+p]
          (M=32 with zero cols so the whole tile is written).
  - Transpose-back: natp -> unT (bf16) -> 4 dense [128,128] bf16 PE
    transposes -> u_nom copied (sample,channel)-interleaved into uxy planes.
  - Barrier math (dCVaR-CBF + closed-form QP) in fp32 natural layout, split
    across DVE and GpSimd; only the largest-sigma GMM mode is evaluated
    (means equal, sigma monotone in variance, CVaR coeff > 0).
    sqrt via magic-seed rsqrt + 1 Newton step (rel err ~2e-3 << tolerance).
    Last group's barrier runs in per-span chunks to shorten the kernel tail.
  - Output assembled run-major: partition p holds samples 64p..64p+63
    interleaved (x,y) -> 512-byte contiguous runs per partition.

Engine budget per core (CoreSim cost model): ACT ~169us (binding: 10.2k
silu lane-cols/span + per-instr access latency), PE ~137us, DVE ~58us,
GpSimd ~14us, DMA fully hidden. Simulated total ~178us vs 532us for the
previous all-fp32 version.
"""
import sys

sys.path.insert(0, '/opt/trn_rl_repo')

from contextlib import ExitStack

import numpy as np
import ml_dtypes

import concourse.bass as bass  # noqa: F401
import concourse.tile as tile
from concourse import bacc, mybir
from concourse.bass_utils import run_bass_kernel_spmd

N_CORES = 8
B = 1_048_576
NF, H1, H2, NC = 16, 128, 32, 2
S = B // N_CORES              # samples per core
SAFE_DIST = 0.8
ALPHA = 2.0
CVAR_COEFF = 1.7549833193248685
SIG_MAX_VAR = 0.3 * 0.3       # largest GMM mode variance (worst-case mode)
EPS_SIG = 1e-8
EPS_DIV = 1e-12

TR = 64                        # samples per partition run
V = 128 * TR                   # natural span = 8192 samples
GRP = 4                        # spans per barrier group
FP32 = mybir.dt.float32
F32R = mybir.dt.float32r
BF16 = mybir.dt.bfloat16
I32 = mybir.dt.int32
BF = ml_dtypes.bfloat16

_cached = {}


def build(s_samples=S, n_devices=N_CORES, act_func=None):
    nc = bacc.Bacc("TRN2", target_bir_lowering=False, debug=False,
                   num_devices=n_devices)
    obs_ap = nc.dram_tensor("obs", [s_samples, NF], F32R, kind="ExternalInput").ap()
    # blobF (f32r): identf(128) | w1pad0(128) | w1pad1(128)
    # blobB (bf16): identb(128) | w2rep(128) | w3stack(32)
    # blobS (fp32): b1c(1) | b2rep(1) | b3c(2)
    bl1_ap = nc.dram_tensor("blobF", [128, 384], F32R, kind="ExternalInput").ap()
    bl2_ap = nc.dram_tensor("blobB", [128, 288], BF16, kind="ExternalInput").ap()
    bl3_ap = nc.dram_tensor("blobS", [128, 4], FP32, kind="ExternalInput").ap()
    out_ap = nc.dram_tensor("out", [s_samples, NC], FP32, kind="ExternalOutput").ap()

    with tile.TileContext(nc) as tc, ExitStack() as ctx:
        kernel_body(ctx, tc, out_ap, obs_ap, bl1_ap, bl2_ap, bl3_ap,
                    s_samples, act_func)
    nc.compile()
    return nc


def kernel_body(ctx, tc, out_ap, obs_ap, bl1_ap, bl2_ap, bl3_ap,
                s_samples, act_func=None):
    nc = tc.nc
    nspan = s_samples // V
    ngrp = nspan // GRP
    SILU = act_func or mybir.ActivationFunctionType.Silu
    ALU = mybir.AluOpType

    const = ctx.enter_context(tc.tile_pool(name="const", bufs=1))
    nat_pool = ctx.enter_context(tc.tile_pool(name="nat", bufs=2))
    obsT_pool = ctx.enter_context(tc.tile_pool(name="obsT", bufs=2))
    y1_pool = ctx.enter_context(tc.tile_pool(name="y1", bufs=2))
    y2_pool = ctx.enter_context(tc.tile_pool(name="y2", bufs=2))
    unT_pool = ctx.enter_context(tc.tile_pool(name="unT", bufs=2))
    plane_pool = ctx.enter_context(tc.tile_pool(name="plane", bufs=2))
    scr_pool = ctx.enter_context(tc.tile_pool(name="scr", bufs=1))
    outb_pool = ctx.enter_context(tc.tile_pool(name="outb", bufs=2))

    ps_l1 = ctx.enter_context(tc.tile_pool(name="ps_l1", bufs=2, space="PSUM"))
    ps_y2 = ctx.enter_context(tc.tile_pool(name="ps_y2", bufs=1, space="PSUM"))
    # tp halves, the 4th L2 group, natp and t2p all rotate through one
    # single-bank ring (their uses are sequential within a span)
    ps_mis = ctx.enter_context(tc.tile_pool(name="ps_mis", bufs=1, space="PSUM"))

    # constants: packed blobs (one DMA per dtype class)
    blobF = const.tile([128, 384], F32R)
    blobB = const.tile([128, 288], BF16)
    blobS = const.tile([128, 4], FP32)
    nc.sync.dma_start(blobF[:], bl1_ap[:])
    identF = blobF[:, 0:128]
    w1pads = (blobF[:, 128:256], blobF[:, 256:384])
    identB = blobB[:, 0:128]
    w2rep = blobB[:, 128:256]
    w3stk = blobB[:, 256:288]
    b1c = blobS[:, 0:1]
    b2rep = blobS[:, 1:2]
    b3c = blobS[:, 2:4]

    V_ = nc.vector
    G_ = nc.gpsimd

    planes = {}

    def group_tiles(gi):
        if gi not in planes:
            planes[gi] = {
                nm: plane_pool.tile([128, GRP * TR * (2 if nm == "uxy" else 1)],
                                    FP32, tag=nm, name=nm)
                for nm in ("relx", "rely", "hvx", "hvy", "uxy")}
        return planes[gi]

    def emit_load(span, split=False):
        obs_nat = nat_pool.tile([128, TR * NF], F32R, tag="obs_nat")
        base = span * V
        src = obs_ap[base:base + V, :].rearrange("(p t) f -> p (t f)", p=128)
        if split:
            # parallel queues at startup: halves on DVE/ACT queues while SP
            # carries the const blobs
            nc.scalar.dma_start(obs_nat[:, 0:512], src[:, 0:512])
            nc.sync.dma_start(obs_nat[:, 512:1024], src[:, 512:1024])
        else:
            nc.sync.dma_start(obs_nat[:], src)
        return obs_nat

    def emit_t1(span, obs_nat):
        gi, sl = span // GRP, span % GRP
        t = group_tiles(gi)
        # barrier input extraction (GpSimd; SBUF->SBUF)
        ob3 = obs_nat[:].rearrange("p (t f) -> p t f", f=NF)
        pl_sl = slice(sl * TR, (sl + 1) * TR)
        G_.tensor_copy(t["relx"][:, pl_sl], ob3[:, :, 6])
        G_.tensor_copy(t["rely"][:, pl_sl], ob3[:, :, 7])
        G_.tensor_copy(t["hvx"][:, pl_sl], ob3[:, :, 8])
        G_.tensor_copy(t["hvy"][:, pl_sl], ob3[:, :, 9])
        # T1: PE transpose natural -> packed obsT (rows 16*(t%8)+f)
        obsT = obsT_pool.tile([128, 1024], F32R, tag="obsT")
        for half in range(2):
            tp = ps_mis.tile([128, 512], F32R, tag="mis", name="tp")
            for ci in range(4):
                c = half * 4 + ci
                nc.tensor.matmul(
                    tp[:, ci * 128:(ci + 1) * 128],
                    obs_nat[:, c * 128:(c + 1) * 128],
                    identF, is_transpose=True)
            V_.tensor_copy(obsT[:, half * 512:(half + 1) * 512], tp[:])
        return obsT

    def emit_barrier(g, tg, nchunks=1):
        P_ = G_
        # ======== barrier math (natural layout, fp32, per group) ========
        # nchunks>1 splits the group column-wise so early chunks' chains can
        # run during later spans' MLP (used for the last group to cut tail).
        Wg = GRP * TR
        Wc = Wg // nchunks
        outb = outb_pool.tile([128, GRP * 2 * TR], FP32, tag="outb")
        for c in range(nchunks):
            ps = slice(c * Wc, (c + 1) * Wc)
            us = slice(c * 2 * Wc, (c + 1) * 2 * Wc)
            relx, rely = tg["relx"][:, ps], tg["rely"][:, ps]
            hvx, hvy = tg["hvx"][:, ps], tg["hvy"][:, ps]
            xv = tg["uxy"][:, us].rearrange("p (w c) -> p w c", c=2)
            ux, uy = xv[:, :, 0], xv[:, :, 1]
            ov = outb[:, us].rearrange("p (w c) -> p w c", c=2)

            a = scr_pool.tile([128, Wc], FP32, tag="a", name="a")
            b_ = scr_pool.tile([128, Wc], FP32, tag="b", name="b_")
            rnsq = scr_pool.tile([128, Wc], FP32, tag="rnsq", name="rnsq")
            rdm = scr_pool.tile([128, Wc], FP32, tag="rdm", name="rdm")
            sig = scr_pool.tile([128, Wc], FP32, tag="sig", name="sig")
            yv = scr_pool.tile([128, Wc], FP32, tag="yv", name="yv")
            k = scr_pool.tile([128, Wc], FP32, tag="k", name="k")
            w = scr_pool.tile([128, Wc], FP32, tag="w", name="w")

            # u_nom += b3 (in place; feeds the dot product and the output)
            V_.tensor_scalar(ux, ux, b3c[:, 0:1], None, ALU.add)
            V_.tensor_scalar(uy, uy, b3c[:, 1:2], None, ALU.add)
            # rnsq, rdm (GpSimd)
            P_.tensor_mul(a[:], relx, relx)
            P_.tensor_mul(b_[:], rely, rely)
            P_.tensor_add(rnsq[:], a[:], b_[:])
            P_.tensor_mul(a[:], hvx, relx)
            P_.tensor_mul(b_[:], hvy, rely)
            P_.tensor_add(rdm[:], a[:], b_[:])      # rel_dot_mu / 2
            # sigma = sqrt(x), x = 4*var*rnsq + eps_sig: magic rsqrt + 1 NR
            V_.tensor_scalar(sig[:], rnsq[:], 4.0 * SIG_MAX_VAR, EPS_SIG,
                             ALU.mult, ALU.add)
            V_.tensor_copy(a[:], sig[:].bitcast(I32))          # f = float(i)
            V_.tensor_scalar(a[:], a[:], -0.5, 1597463007.0, ALU.mult, ALU.add)
            V_.tensor_copy(yv[:].bitcast(I32), a[:])           # y0 bits
            P_.tensor_mul(a[:], yv[:], yv[:])
            P_.tensor_mul(a[:], a[:], sig[:])
            V_.tensor_scalar(a[:], a[:], -0.5, 1.5, ALU.mult, ALU.add)
            P_.tensor_mul(yv[:], yv[:], a[:])
            V_.tensor_mul(sig[:], sig[:], yv[:])               # sqrt = x*rsqrt
            # k = rdm - rel.u - rnsq + CV/2*sigma ; viol/2 = k + S^2
            P_.tensor_mul(a[:], relx, ux)
            P_.tensor_mul(b_[:], rely, uy)
            V_.tensor_add(a[:], a[:], b_[:])
            V_.tensor_sub(k[:], rdm[:], a[:])
            V_.tensor_sub(k[:], k[:], rnsq[:])
            V_.scalar_tensor_tensor(k[:], sig[:], 0.5 * CVAR_COEFF, k[:],
                                    ALU.mult, ALU.add)
            # v = max(viol/2, 0) ; w = gnsq/2 = 2*rnsq + eps/2
            V_.tensor_scalar(k[:], k[:], SAFE_DIST ** 2, 0.0, ALU.add, ALU.max)
            V_.tensor_scalar(w[:], rnsq[:], 2.0, 0.5 * EPS_DIV,
                             ALU.mult, ALU.add)
            V_.reciprocal(w[:], w[:])
            V_.tensor_mul(k[:], k[:], w[:])                    # coef
            # u_safe = u + 2*coef*rel
            P_.tensor_mul(a[:], k[:], relx)
            V_.scalar_tensor_tensor(ov[:, :, 0], a[:], 2.0, ux,
                                    ALU.mult, ALU.add)
            P_.tensor_mul(b_[:], k[:], rely)
            V_.scalar_tensor_tensor(ov[:, :, 1], b_[:], 2.0, uy,
                                    ALU.mult, ALU.add)

            # store run-major for the spans this chunk fully covers
            spans_per_chunk = GRP // nchunks
            for s3 in range(spans_per_chunk):
                s2 = c * spans_per_chunk + s3
                base = (g * GRP + s2) * V
                dst = out_ap[base:base + V, :].rearrange(
                    "(p t) c -> p (t c)", p=128)
                nc.sync.dma_start(dst, outb[:, s2 * 2 * TR:(s2 + 1) * 2 * TR])

    obs_nxt = emit_load(0, split=True)
    nc.sync.dma_start(blobS[:], bl3_ap[:])
    nc.sync.dma_start(blobB[:], bl2_ap[:])
    obsT_nxt = emit_t1(0, obs_nxt)

    for span in range(nspan):
        g, sl = span // GRP, span % GRP
        obsT = obsT_nxt
        tg = group_tiles(g)
        uxy = tg["uxy"]

        # layouts: obsT col = c*128 + p; row = 16*(t%8) + f; c = t//8.
        # y1sT col = (t8*2 + h)*512 + c4*128 + p   (h: half, c4 = c%4)
        # y2sT col = hs*512 + c4*128 + p, hs = 2h+sub, groups t8 = 4*sub+j
        y1sT = y1_pool.tile([128, 8192], BF16, tag="y1sT")
        y2sT = y2_pool.tile([128, 2048], BF16, tag="y2sT")

        # ---- L1: 2 concurrent row-tiles per [128,1024] psum tile ----
        for h in range(2):
            hs_cols = slice(h * 512, (h + 1) * 512)
            for par in range(2):
                for qh in range(2):
                    l1 = ps_l1.tile([128, 1024], FP32, tag="l1")
                    for qq in range(2):
                        q = 2 * qh + qq
                        nc.tensor.matmul(
                            l1[:, qq * 512:(qq + 1) * 512],
                            w1pads[par][32 * q:32 * q + 32, :],
                            obsT[32 * q:32 * q + 32, hs_cols],
                            start=True, stop=True,
                            tile_position=(32 * q, 0))
                    t80 = 4 * qh + par
                    dst = y1sT[:].rearrange("p (t8 h2 n) -> p t8 h2 n",
                                            t8=8, h2=2)[:, t80:t80 + 3:2, h]
                    srcv = l1[:].rearrange("p (q n) -> p q n", q=2)
                    nc.scalar.activation(dst, srcv, SILU,
                                         bias=b1c[:, 0:1], scale=1.0)

        # ---- pipeline next span's load + extracts + T1 ----
        if span + 1 < nspan:
            obs_nxt = emit_load(span + 1)
            obsT_nxt = emit_t1(span + 1, obs_nxt)

        # ---- L2: col-tiled K=128; hs 0-2 share one act, hs 3 via ring ----
        y2big = ps_y2.tile([128, 1536], FP32, tag="y2p")
        for hs in range(3):
            h, sub = hs // 2, hs % 2
            for j in range(4):
                t8 = 4 * sub + j
                nc.tensor.matmul(
                    y2big[32 * j:32 * j + 32, hs * 512:(hs + 1) * 512],
                    w2rep[:, 32 * j:32 * j + 32],
                    y1sT[:, (t8 * 2 + h) * 512:(t8 * 2 + h + 1) * 512],
                    start=True, stop=True,
                    tile_position=(0, 32 * j))
        nc.scalar.activation(y2sT[:, 0:1536], y2big[:], SILU,
                             bias=b2rep[:, 0:1], scale=1.0)
        y2d = ps_mis.tile([128, 512], FP32, tag="mis", name="y2d")
        for j in range(4):
            t8 = 4 + j
            nc.tensor.matmul(
                y2d[32 * j:32 * j + 32, :],
                w2rep[:, 32 * j:32 * j + 32],
                y1sT[:, (t8 * 2 + 1) * 512:(t8 * 2 + 2) * 512],
                start=True, stop=True,
                tile_position=(0, 32 * j))
        nc.scalar.activation(y2sT[:, 1536:2048], y2d[:], SILU,
                             bias=b2rep[:, 0:1], scale=1.0)

        # ---- L3: col-tiled block-diag w3 -> dense natural-ish psum ----
        # natp row = 32*hs + 2*j + ch, col = c4*128 + p
        natp = ps_mis.tile([128, 512], FP32, tag="mis", name="natp")
        for hs in range(4):
            nc.tensor.matmul(
                natp[32 * hs:32 * hs + 32, :],
                w3stk[:],
                y2sT[:, hs * 512:(hs + 1) * 512],
                start=True, stop=True,
                tile_position=(0, 32 * hs))
        unTt = unT_pool.tile([128, 512], BF16, tag="unT")
        V_.tensor_copy(unTt[:], natp[:])

        # ---- T2: dense transpose-back [128,128] blocks ----
        t2p = ps_mis.tile([128, 512], BF16, tag="mis", name="t2p")
        for c4 in range(4):
            nc.tensor.matmul(
                t2p[:, c4 * 128:(c4 + 1) * 128],
                unTt[:, c4 * 128:(c4 + 1) * 128],
                identB[:], is_transpose=True)
        # t2p col = c4*128 + 64h+32sub+(2j+ch) ; sample t = 32h+8c4+4sub+j
        # uxy col = sl*128 + 2t+ch = sl*128 + 64h+16c4+8sub+(2j+ch)
        inv = t2p[:].rearrange("p (c4 h sub jc) -> p h c4 sub jc",
                               c4=4, h=2, sub=2)[:, :, :, :, 0:8]
        outv = uxy[:, sl * 128:(sl + 1) * 128].rearrange(
            "p (h c4 sub jc) -> p h c4 sub jc", h=2, c4=4, sub=2)
        V_.tensor_copy(outv, inv)

        if sl == GRP - 1:
            emit_barrier(g, tg, nchunks=GRP if g == ngrp - 1 else 1)
            del planes[g]


def prep_consts(w1, b1, w2, b2, w3, b3):
    w1pad0 = np.zeros((128, 128), np.float32)
    w1pad1 = np.zeros((128, 128), np.float32)
    w2rep = np.zeros((128, 128), BF)
    w3stack = np.zeros((128, 32), BF)
    for q in range(4):
        w1pad0[32 * q:32 * q + 16, :] = w1.T          # even t8 groups
        w1pad1[32 * q + 16:32 * q + 32, :] = w1.T     # odd t8 groups
    for j in range(4):
        w2rep[:, 32 * j:32 * j + 32] = w2.T.astype(BF)
        w3stack[32 * j:32 * j + 32, 2 * j:2 * j + 2] = w3.T.astype(BF)
    b3c = np.empty((128, 2), np.float32)
    b3c[:, 0] = b3[0]
    b3c[:, 1] = b3[1]
    blobF = np.concatenate([np.eye(128, dtype=np.float32), w1pad0, w1pad1],
                           axis=1)
    blobB = np.concatenate([np.eye(128, dtype=BF), w2rep, w3stack], axis=1)
    blobS = np.concatenate([
        np.asarray(b1, np.float32).reshape(128, 1),
        np.tile(np.asarray(b2, np.float32), 4).reshape(128, 1),
        b3c], axis=1)
    return dict(blobF=np.ascontiguousarray(blobF),
                blobB=np.ascontiguousarray(blobB),
                blobS=np.ascontiguousarray(blobS))


def kernel(obs, w1, b1, w2, b2, w3, b3):
    obs = np.asarray(obs, np.float32)
    consts = prep_consts(np.asarray(w1, np.float32), np.asarray(b1, np.float32),
                         np.asarray(w2, np.float32), np.asarray(b2, np.float32),
                         np.asarray(w3, np.float32), np.asarray(b3, np.float32))
    if "nc" not in _cached:
        _cached["nc"] = build()
    nc = _cached["nc"]
    in_maps = []
    for c in range(N_CORES):
        m = {"obs": np.ascontiguousarray(obs[c * S:(c + 1) * S])}
        m.update(consts)
        in_maps.append(m)
    res = run_bass_kernel_spmd(nc, in_maps, list(range(N_CORES)))
    out = np.empty((B, NC), np.float32)
    for c in range(N_CORES):
        out[c * S:(c + 1) * S] = res.results[c]["out"]
    return out


# revision 54
# speedup vs baseline: 1.4532x; 1.0362x over previous
"""BarrierNet Trainium2 kernel: 8-core data-parallel Bass/Tile implementation.

Takes full inputs, shards batch across 8 NeuronCores, returns full output.

Per-core structure (S = 131072 samples, 16 spans of 8192):
  - obs loaded naturally as f32r: partition p of a span holds samples
    [base+64p, base+64p+64) (4KB contiguous per partition -> full DMA
    efficiency); next span's load/extract/transpose pipelined one span ahead.
  - T1: PE block-transposes [128,128] blocks into packed obsT (rows
    16*(t%8)+f), all-f32r (moving identity f32r: 1.5 cyc/row).
  - MLP with all matmul moving operands at full streaming rate:
      L1: K=32 zero-padded w1 strips (f32r, 1 cyc/row), 2 concurrent
          row-tiles per [128,1024] PSUM tile (bufs=2 -> ACT/PE overlap);
          silu via ACT -> y1sT bf16.
      L2: col-tiled K=128 (w2 bf16, 1 cyc/row); hs groups 0-2 drain through
          one [128,1536] ACT silu, group 3 via the shared psum ring -> y2sT
          bf16.
      L3: col-tiled block-diag w3 (bf16) -> dense natp[32*hs+2j+ch, c4*128+p]
          (M=32 with zero cols so the whole tile is written).
  - Transpose-back: natp -> unT (bf16) -> 4 dense [128,128] bf16 PE
    transposes -> u_nom copied (sample,channel)-interleaved into uxy planes.
  - Barrier math (dCVaR-CBF + closed-form QP) in fp32 natural layout, split
    across DVE and GpSimd; only the largest-sigma GMM mode is evaluated
    (means equal, sigma monotone in variance, CVaR coeff > 0).
    sqrt via magic-seed rsqrt + 1 Newton step (rel err ~2e-3 << tolerance).
    Last group's barrier runs in per-span chunks to shorten the kernel tail.
  - Output assembled run-major: partition p holds samples 64p..64p+63
    interleaved (x,y) -> 512-byte contiguous runs per partition.

Engine budget per core (CoreSim cost model): ACT ~169us (binding: 10.2k
silu lane-cols/span + per-instr access latency), PE ~137us, DVE ~58us,
GpSimd ~14us, DMA fully hidden. Simulated total ~177.4us vs 532us for the
previous all-fp32 version.
"""
import sys

sys.path.insert(0, '/opt/trn_rl_repo')

from contextlib import ExitStack

import numpy as np
import ml_dtypes

import concourse.bass as bass  # noqa: F401
import concourse.tile as tile
from concourse import bacc, mybir
from concourse.bass_utils import run_bass_kernel_spmd

N_CORES = 8
B = 1_048_576
NF, H1, H2, NC = 16, 128, 32, 2
S = B // N_CORES              # samples per core
SAFE_DIST = 0.8
ALPHA = 2.0
CVAR_COEFF = 1.7549833193248685
SIG_MAX_VAR = 0.3 * 0.3       # largest GMM mode variance (worst-case mode)
EPS_SIG = 1e-8
EPS_DIV = 1e-12

TR = 64                        # samples per partition run
V = 128 * TR                   # natural span = 8192 samples
GRP = 4                        # spans per barrier group
FP32 = mybir.dt.float32
F32R = mybir.dt.float32r
BF16 = mybir.dt.bfloat16
I32 = mybir.dt.int32
BF = ml_dtypes.bfloat16

_cached = {}


def build(s_samples=S, n_devices=N_CORES, act_func=None):
    nc = bacc.Bacc("TRN2", target_bir_lowering=False, debug=False,
                   num_devices=n_devices)
    obs_ap = nc.dram_tensor("obs", [s_samples, NF], F32R, kind="ExternalInput").ap()
    # blobF (f32r): identf(128) | w1pad0(128) | w1pad1(128)
    # blobB (bf16): identb(128) | w2rep(128) | w3stack(32)
    # blobS (fp32): b1c(1) | b2rep(1) | b3c(2)
    bl1_ap = nc.dram_tensor("blobF", [128, 384], F32R, kind="ExternalInput").ap()
    bl2_ap = nc.dram_tensor("blobB", [128, 288], BF16, kind="ExternalInput").ap()
    bl3_ap = nc.dram_tensor("blobS", [128, 4], FP32, kind="ExternalInput").ap()
    out_ap = nc.dram_tensor("out", [s_samples, NC], FP32, kind="ExternalOutput").ap()

    with tile.TileContext(nc) as tc, ExitStack() as ctx:
        kernel_body(ctx, tc, out_ap, obs_ap, bl1_ap, bl2_ap, bl3_ap,
                    s_samples, act_func)
    nc.compile()
    return nc


def kernel_body(ctx, tc, out_ap, obs_ap, bl1_ap, bl2_ap, bl3_ap,
                s_samples, act_func=None):
    nc = tc.nc
    nspan = s_samples // V
    ngrp = nspan // GRP
    SILU = act_func or mybir.ActivationFunctionType.Silu
    ALU = mybir.AluOpType

    const = ctx.enter_context(tc.tile_pool(name="const", bufs=1))
    nat_pool = ctx.enter_context(tc.tile_pool(name="nat", bufs=2))
    obsT_pool = ctx.enter_context(tc.tile_pool(name="obsT", bufs=2))
    y1_pool = ctx.enter_context(tc.tile_pool(name="y1", bufs=2))
    y2_pool = ctx.enter_context(tc.tile_pool(name="y2", bufs=2))
    unT_pool = ctx.enter_context(tc.tile_pool(name="unT", bufs=2))
    plane_pool = ctx.enter_context(tc.tile_pool(name="plane", bufs=2))
    scr_pool = ctx.enter_context(tc.tile_pool(name="scr", bufs=1))
    outb_pool = ctx.enter_context(tc.tile_pool(name="outb", bufs=2))

    ps_l1 = ctx.enter_context(tc.tile_pool(name="ps_l1", bufs=2, space="PSUM"))
    ps_y2 = ctx.enter_context(tc.tile_pool(name="ps_y2", bufs=1, space="PSUM"))
    # tp halves, the 4th L2 group, natp and t2p all rotate through one
    # single-bank ring (their uses are sequential within a span)
    ps_mis = ctx.enter_context(tc.tile_pool(name="ps_mis", bufs=1, space="PSUM"))

    # constants: packed blobs (one DMA per dtype class)
    blobF = const.tile([128, 384], F32R)
    blobB = const.tile([128, 288], BF16)
    blobS = const.tile([128, 4], FP32)
    nc.sync.dma_start(blobF[:], bl1_ap[:])
    identF = blobF[:, 0:128]
    w1pads = (blobF[:, 128:256], blobF[:, 256:384])
    identB = blobB[:, 0:128]
    w2rep = blobB[:, 128:256]
    w3stk = blobB[:, 256:288]
    b1c = blobS[:, 0:1]
    b2rep = blobS[:, 1:2]
    b3c = blobS[:, 2:4]

    V_ = nc.vector
    G_ = nc.gpsimd

    planes = {}

    def group_tiles(gi):
        if gi not in planes:
            planes[gi] = {
                nm: plane_pool.tile([128, GRP * TR * (2 if nm == "uxy" else 1)],
                                    FP32, tag=nm, name=nm)
                for nm in ("relx", "rely", "hvx", "hvy", "uxy")}
        return planes[gi]

    def emit_load(span, split=False):
        obs_nat = nat_pool.tile([128, TR * NF], F32R, tag="obs_nat")
        base = span * V
        src = obs_ap[base:base + V, :].rearrange("(p t) f -> p (t f)", p=128)
        if split:
            # parallel queues at startup: halves on DVE/ACT queues while SP
            # carries the const blobs
            nc.scalar.dma_start(obs_nat[:, 0:512], src[:, 0:512])
            nc.sync.dma_start(obs_nat[:, 512:1024], src[:, 512:1024])
        else:
            nc.sync.dma_start(obs_nat[:], src)
        return obs_nat

    def emit_t1(span, obs_nat):
        gi, sl = span // GRP, span % GRP
        t = group_tiles(gi)
        # barrier input extraction (GpSimd; SBUF->SBUF)
        ob3 = obs_nat[:].rearrange("p (t f) -> p t f", f=NF)
        pl_sl = slice(sl * TR, (sl + 1) * TR)
        G_.tensor_copy(t["relx"][:, pl_sl], ob3[:, :, 6])
        G_.tensor_copy(t["rely"][:, pl_sl], ob3[:, :, 7])
        G_.tensor_copy(t["hvx"][:, pl_sl], ob3[:, :, 8])
        G_.tensor_copy(t["hvy"][:, pl_sl], ob3[:, :, 9])
        # T1: PE transpose natural -> packed obsT (rows 16*(t%8)+f)
        obsT = obsT_pool.tile([128, 1024], F32R, tag="obsT")
        for half in range(2):
            tp = ps_mis.tile([128, 512], F32R, tag="mis", name="tp")
            for ci in range(4):
                c = half * 4 + ci
                nc.tensor.matmul(
                    tp[:, ci * 128:(ci + 1) * 128],
                    obs_nat[:, c * 128:(c + 1) * 128],
                    identF, is_transpose=True)
            V_.tensor_copy(obsT[:, half * 512:(half + 1) * 512], tp[:])
        return obsT

    def emit_barrier(g, tg, nchunks=1):
        P_ = G_
        # ======== barrier math (natural layout, fp32, per group) ========
        # nchunks>1 splits the group column-wise so early chunks' chains can
        # run during later spans' MLP (used for the last group to cut tail).
        Wg = GRP * TR
        Wc = Wg // nchunks
        outb = outb_pool.tile([128, GRP * 2 * TR], FP32, tag="outb")
        for c in range(nchunks):
            ps = slice(c * Wc, (c + 1) * Wc)
            us = slice(c * 2 * Wc, (c + 1) * 2 * Wc)
            relx, rely = tg["relx"][:, ps], tg["rely"][:, ps]
            hvx, hvy = tg["hvx"][:, ps], tg["hvy"][:, ps]
            xv = tg["uxy"][:, us].rearrange("p (w c) -> p w c", c=2)
            ux, uy = xv[:, :, 0], xv[:, :, 1]
            ov = outb[:, us].rearrange("p (w c) -> p w c", c=2)

            a = scr_pool.tile([128, Wc], FP32, tag="a", name="a")
            b_ = scr_pool.tile([128, Wc], FP32, tag="b", name="b_")
            rnsq = scr_pool.tile([128, Wc], FP32, tag="rnsq", name="rnsq")
            rdm = scr_pool.tile([128, Wc], FP32, tag="rdm", name="rdm")
            sig = scr_pool.tile([128, Wc], FP32, tag="sig", name="sig")
            yv = scr_pool.tile([128, Wc], FP32, tag="yv", name="yv")
            k = scr_pool.tile([128, Wc], FP32, tag="k", name="k")
            w = scr_pool.tile([128, Wc], FP32, tag="w", name="w")

            # u_nom += b3 (in place; feeds the dot product and the output)
            V_.tensor_scalar(ux, ux, b3c[:, 0:1], None, ALU.add)
            V_.tensor_scalar(uy, uy, b3c[:, 1:2], None, ALU.add)
            # rnsq, rdm (GpSimd)
            P_.tensor_mul(a[:], relx, relx)
            P_.tensor_mul(b_[:], rely, rely)
            P_.tensor_add(rnsq[:], a[:], b_[:])
            P_.tensor_mul(a[:], hvx, relx)
            P_.tensor_mul(b_[:], hvy, rely)
            P_.tensor_add(rdm[:], a[:], b_[:])      # rel_dot_mu / 2
            # sigma = sqrt(x), x = 4*var*rnsq + eps_sig: magic rsqrt + 1 NR
            V_.tensor_scalar(sig[:], rnsq[:], 4.0 * SIG_MAX_VAR, EPS_SIG,
                             ALU.mult, ALU.add)
            V_.tensor_copy(a[:], sig[:].bitcast(I32))          # f = float(i)
            V_.tensor_scalar(a[:], a[:], -0.5, 1597463007.0, ALU.mult, ALU.add)
            V_.tensor_copy(yv[:].bitcast(I32), a[:])           # y0 bits
            P_.tensor_mul(a[:], yv[:], yv[:])
            P_.tensor_mul(a[:], a[:], sig[:])
            V_.tensor_scalar(a[:], a[:], -0.5, 1.5, ALU.mult, ALU.add)
            P_.tensor_mul(yv[:], yv[:], a[:])
            V_.tensor_mul(sig[:], sig[:], yv[:])               # sqrt = x*rsqrt
            # Precompute (rel-only, runs during MLP): P = rdm - rnsq
            # + CV/2*sigma + S^2 ; r = 1/(2*rnsq + eps/2). Post-u_nom chain
            # is then just: k = P - rel.u ; coef = max(k,0)*r ;
            # out = u + 2*coef*rel.
            V_.tensor_sub(rdm[:], rdm[:], rnsq[:])
            V_.scalar_tensor_tensor(rdm[:], sig[:], 0.5 * CVAR_COEFF, rdm[:],
                                    ALU.mult, ALU.add)
            V_.tensor_scalar(rdm[:], rdm[:], SAFE_DIST ** 2, None, ALU.add)
            V_.tensor_scalar(w[:], rnsq[:], 2.0, 0.5 * EPS_DIV,
                             ALU.mult, ALU.add)
            V_.reciprocal(w[:], w[:])
            # ---- post-u_nom chain ----
            P_.tensor_mul(a[:], relx, ux)
            P_.tensor_mul(b_[:], rely, uy)
            V_.tensor_add(a[:], a[:], b_[:])
            V_.tensor_sub(k[:], rdm[:], a[:])
            V_.tensor_scalar(k[:], k[:], 0.0, None, ALU.max)
            V_.tensor_mul(k[:], k[:], w[:])                    # coef
            # u_safe = u + 2*coef*rel
            P_.tensor_mul(a[:], k[:], relx)
            V_.scalar_tensor_tensor(ov[:, :, 0], a[:], 2.0, ux,
                                    ALU.mult, ALU.add)
            P_.tensor_mul(b_[:], k[:], rely)
            V_.scalar_tensor_tensor(ov[:, :, 1], b_[:], 2.0, uy,
                                    ALU.mult, ALU.add)

            # store run-major for the spans this chunk fully covers
            spans_per_chunk = GRP // nchunks
            for s3 in range(spans_per_chunk):
                s2 = c * spans_per_chunk + s3
                base = (g * GRP + s2) * V
                dst = out_ap[base:base + V, :].rearrange(
                    "(p t) c -> p (t c)", p=128)
                nc.sync.dma_start(dst, outb[:, s2 * 2 * TR:(s2 + 1) * 2 * TR])

    obs_nxt = emit_load(0, split=True)
    nc.sync.dma_start(blobS[:], bl3_ap[:])
    nc.sync.dma_start(blobB[:], bl2_ap[:])
    obsT_nxt = emit_t1(0, obs_nxt)

    for span in range(nspan):
        g, sl = span // GRP, span % GRP
        obsT = obsT_nxt
        tg = group_tiles(g)
        uxy = tg["uxy"]

        # layouts: obsT col = c*128 + p; row = 16*(t%8) + f; c = t//8.
        # y1sT col = (t8*2 + h)*512 + c4*128 + p   (h: half, c4 = c%4)
        # y2sT col = hs*512 + c4*128 + p, hs = 2h+sub, groups t8 = 4*sub+j
        y1sT = y1_pool.tile([128, 8192], BF16, tag="y1sT")
        y2sT = y2_pool.tile([128, 2048], BF16, tag="y2sT")

        # ---- L1: 2 concurrent row-tiles per [128,1024] psum tile ----
        for h in range(2):
            hs_cols = slice(h * 512, (h + 1) * 512)
            for par in range(2):
                for qh in range(2):
                    l1 = ps_l1.tile([128, 1024], FP32, tag="l1")
                    for qq in range(2):
                        q = 2 * qh + qq
                        nc.tensor.matmul(
                            l1[:, qq * 512:(qq + 1) * 512],
                            w1pads[par][32 * q:32 * q + 32, :],
                            obsT[32 * q:32 * q + 32, hs_cols],
                            start=True, stop=True,
                            tile_position=(32 * q, 0))
                    t80 = 4 * qh + par
                    dst = y1sT[:].rearrange("p (t8 h2 n) -> p t8 h2 n",
                                            t8=8, h2=2)[:, t80:t80 + 3:2, h]
                    srcv = l1[:].rearrange("p (q n) -> p q n", q=2)
                    nc.scalar.activation(dst, srcv, SILU,
                                         bias=b1c[:, 0:1], scale=1.0)

        # ---- pipeline next span's load + extracts + T1 ----
        if span + 1 < nspan:
            obs_nxt = emit_load(span + 1)
            obsT_nxt = emit_t1(span + 1, obs_nxt)

        # ---- L2: col-tiled K=128; hs 0-2 share one act, hs 3 via ring ----
        y2big = ps_y2.tile([128, 1536], FP32, tag="y2p")
        for hs in range(3):
            h, sub = hs // 2, hs % 2
            for j in range(4):
                t8 = 4 * sub + j
                nc.tensor.matmul(
                    y2big[32 * j:32 * j + 32, hs * 512:(hs + 1) * 512],
                    w2rep[:, 32 * j:32 * j + 32],
                    y1sT[:, (t8 * 2 + h) * 512:(t8 * 2 + h + 1) * 512],
                    start=True, stop=True,
                    tile_position=(0, 32 * j))
        if span == nspan - 1:
            # split so L3 hs=0,1 can start during the second act
            nc.scalar.activation(y2sT[:, 0:1024], y2big[:, 0:1024], SILU,
                                 bias=b2rep[:, 0:1], scale=1.0)
            nc.scalar.activation(y2sT[:, 1024:1536], y2big[:, 1024:1536], SILU,
                                 bias=b2rep[:, 0:1], scale=1.0)
        else:
            nc.scalar.activation(y2sT[:, 0:1536], y2big[:], SILU,
                                 bias=b2rep[:, 0:1], scale=1.0)
        y2d = ps_mis.tile([128, 512], FP32, tag="mis", name="y2d")
        for j in range(4):
            t8 = 4 + j
            nc.tensor.matmul(
                y2d[32 * j:32 * j + 32, :],
                w2rep[:, 32 * j:32 * j + 32],
                y1sT[:, (t8 * 2 + 1) * 512:(t8 * 2 + 2) * 512],
                start=True, stop=True,
                tile_position=(0, 32 * j))
        nc.scalar.activation(y2sT[:, 1536:2048], y2d[:], SILU,
                             bias=b2rep[:, 0:1], scale=1.0)

        # ---- L3: col-tiled block-diag w3 -> dense natural-ish psum ----
        # natp row = 32*hs + 2*j + ch, col = c4*128 + p
        natp = ps_mis.tile([128, 512], FP32, tag="mis", name="natp")
        for hs in range(4):
            nc.tensor.matmul(
                natp[32 * hs:32 * hs + 32, :],
                w3stk[:],
                y2sT[:, hs * 512:(hs + 1) * 512],
                start=True, stop=True,
                tile_position=(0, 32 * hs))
        unTt = unT_pool.tile([128, 512], BF16, tag="unT")
        V_.tensor_copy(unTt[:], natp[:])

        # ---- T2: dense transpose-back [128,128] blocks ----
        t2p = ps_mis.tile([128, 512], BF16, tag="mis", name="t2p")
        for c4 in range(4):
            nc.tensor.matmul(
                t2p[:, c4 * 128:(c4 + 1) * 128],
                unTt[:, c4 * 128:(c4 + 1) * 128],
                identB[:], is_transpose=True)
        # t2p col = c4*128 + 64h+32sub+(2j+ch) ; sample t = 32h+8c4+4sub+j
        # uxy col = sl*128 + 2t+ch = sl*128 + 64h+16c4+8sub+(2j+ch)
        inv = t2p[:].rearrange("p (c4 h sub jc) -> p h c4 sub jc",
                               c4=4, h=2, sub=2)[:, :, :, :, 0:8]
        outv = uxy[:, sl * 128:(sl + 1) * 128].rearrange(
            "p (h c4 sub jc) -> p h c4 sub jc", h=2, c4=4, sub=2)
        V_.tensor_copy(outv, inv)

        if sl == GRP - 1:
            emit_barrier(g, tg, nchunks=GRP if g == ngrp - 1 else 1)
            del planes[g]


def prep_consts(w1, b1, w2, b2, w3, b3):
    w1pad0 = np.zeros((128, 128), np.float32)
    w1pad1 = np.zeros((128, 128), np.float32)
    w2rep = np.zeros((128, 128), BF)
    w3stack = np.zeros((128, 32), BF)
    for q in range(4):
        w1pad0[32 * q:32 * q + 16, :] = w1.T          # even t8 groups
        w1pad1[32 * q + 16:32 * q + 32, :] = w1.T     # odd t8 groups
    for j in range(4):
        w2rep[:, 32 * j:32 * j + 32] = w2.T.astype(BF)
        w3stack[32 * j:32 * j + 32, 2 * j:2 * j + 2] = w3.T.astype(BF)
    b3c = np.empty((128, 2), np.float32)
    b3c[:, 0] = b3[0]
    b3c[:, 1] = b3[1]
    blobF = np.concatenate([np.eye(128, dtype=np.float32), w1pad0, w1pad1],
                           axis=1)
    blobB = np.concatenate([np.eye(128, dtype=BF), w2rep, w3stack], axis=1)
    blobS = np.concatenate([
        np.asarray(b1, np.float32).reshape(128, 1),
        np.tile(np.asarray(b2, np.float32), 4).reshape(128, 1),
        b3c], axis=1)
    return dict(blobF=np.ascontiguousarray(blobF),
                blobB=np.ascontiguousarray(blobB),
                blobS=np.ascontiguousarray(blobS))


def kernel(obs, w1, b1, w2, b2, w3, b3):
    obs = np.asarray(obs, np.float32)
    consts = prep_consts(np.asarray(w1, np.float32), np.asarray(b1, np.float32),
                         np.asarray(w2, np.float32), np.asarray(b2, np.float32),
                         np.asarray(w3, np.float32), np.asarray(b3, np.float32))
    if "nc" not in _cached:
        _cached["nc"] = build()
    nc = _cached["nc"]
    in_maps = []
    for c in range(N_CORES):
        m = {"obs": np.ascontiguousarray(obs[c * S:(c + 1) * S])}
        m.update(consts)
        in_maps.append(m)
    res = run_bass_kernel_spmd(nc, in_maps, list(range(N_CORES)))
    out = np.empty((B, NC), np.float32)
    for c in range(N_CORES):
        out[c * S:(c + 1) * S] = res.results[c]["out"]
    return out


# revision 59
# speedup vs baseline: 1.5282x; 1.0516x over previous
"""BarrierNet Trainium2 kernel: 8-core data-parallel Bass/Tile implementation.

Takes full inputs, shards batch across 8 NeuronCores, returns full output.

Per-core structure (S = 131072 samples, 16 spans of 8192):
  - obs loaded naturally as f32r: partition p of a span holds samples
    [base+64p, base+64p+64) (4KB contiguous per partition -> full DMA
    efficiency); next span's load/extract/transpose pipelined one span ahead.
  - T1: PE block-transposes [128,128] blocks into packed obsT (rows
    16*(t%8)+f), all-f32r (moving identity f32r: 1.5 cyc/row).
  - MLP with all matmul moving operands at full streaming rate:
      L1: K=32 zero-padded w1 strips (f32r, 1 cyc/row), 2 concurrent
          row-tiles per [128,1024] PSUM tile (bufs=2 -> ACT/PE overlap);
          silu via ACT -> y1sT bf16.
      L2: col-tiled K=128 (w2 bf16, 1 cyc/row); hs groups 0-2 drain through
          one [128,1536] ACT silu, group 3 via the shared psum ring -> y2sT
          bf16.
      L3: col-tiled block-diag w3 (bf16) -> dense natp[32*hs+2j+ch, c4*128+p]
          (M=32 with zero cols so the whole tile is written).
  - Transpose-back: natp -> unT (bf16) -> 4 dense [128,128] bf16 PE
    transposes -> u_nom copied (sample,channel)-interleaved into uxy planes.
  - Barrier math (dCVaR-CBF + closed-form QP) in fp32 natural layout, split
    across DVE and GpSimd; only the largest-sigma GMM mode is evaluated
    (means equal, sigma monotone in variance, CVaR coeff > 0).
    sqrt via magic-seed rsqrt + 1 Newton step (rel err ~2e-3 << tolerance).
    Last group's barrier runs in per-span chunks to shorten the kernel tail.
  - Output assembled run-major: partition p holds samples 64p..64p+63
    interleaved (x,y) -> 512-byte contiguous runs per partition.

Engine budget per core (CoreSim cost model): ACT ~169us (binding: 10.2k
silu lane-cols/span + per-instr access latency), PE ~137us, DVE ~58us,
GpSimd ~14us, DMA fully hidden. Simulated total ~177.4us vs 532us for the
previous all-fp32 version.
"""
import sys

sys.path.insert(0, '/opt/trn_rl_repo')

from contextlib import ExitStack

import numpy as np
import ml_dtypes

import concourse.bass as bass  # noqa: F401
import concourse.tile as tile
from concourse import bacc, mybir
from concourse.bass_utils import run_bass_kernel_spmd

N_CORES = 8
B = 1_048_576
NF, H1, H2, NC = 16, 128, 32, 2
S = B // N_CORES              # samples per core
SAFE_DIST = 0.8
ALPHA = 2.0
CVAR_COEFF = 1.7549833193248685
SIG_MAX_VAR = 0.3 * 0.3       # largest GMM mode variance (worst-case mode)
EPS_SIG = 1e-8
EPS_DIV = 1e-12

TR = 64                        # samples per partition run
V = 128 * TR                   # natural span = 8192 samples
GRP = 4                        # spans per barrier group
FP32 = mybir.dt.float32
F32R = mybir.dt.float32r
BF16 = mybir.dt.bfloat16
I32 = mybir.dt.int32
BF = ml_dtypes.bfloat16

_cached = {}


def build(s_samples=S, n_devices=N_CORES, act_func=None):
    nc = bacc.Bacc("TRN2", target_bir_lowering=False, debug=False,
                   num_devices=n_devices)
    obs_ap = nc.dram_tensor("obs", [s_samples, NF], F32R, kind="ExternalInput").ap()
    # blobF (f32r): identf(128) | w1pad0(128) | w1pad1(128)
    # blobB (bf16): identb(128) | w2rep(128) | w3stack(32)
    # blobS (fp32): b1c(1) | b2rep(1) | b3c(2)
    bl1_ap = nc.dram_tensor("blobF", [128, 384], F32R, kind="ExternalInput").ap()
    bl2_ap = nc.dram_tensor("blobB", [128, 288], BF16, kind="ExternalInput").ap()
    bl3_ap = nc.dram_tensor("blobS", [128, 4], FP32, kind="ExternalInput").ap()
    out_ap = nc.dram_tensor("out", [s_samples, NC], FP32, kind="ExternalOutput").ap()

    with tile.TileContext(nc) as tc, ExitStack() as ctx:
        kernel_body(ctx, tc, out_ap, obs_ap, bl1_ap, bl2_ap, bl3_ap,
                    s_samples, act_func)
    nc.compile()
    return nc


def kernel_body(ctx, tc, out_ap, obs_ap, bl1_ap, bl2_ap, bl3_ap,
                s_samples, act_func=None):
    nc = tc.nc
    nspan = s_samples // V
    ngrp = nspan // GRP
    SILU = act_func or mybir.ActivationFunctionType.Silu
    ALU = mybir.AluOpType

    const = ctx.enter_context(tc.tile_pool(name="const", bufs=1))
    nat_pool = ctx.enter_context(tc.tile_pool(name="nat", bufs=2))
    obsT_pool = ctx.enter_context(tc.tile_pool(name="obsT", bufs=2))
    y1_pool = ctx.enter_context(tc.tile_pool(name="y1", bufs=2))
    y2_pool = ctx.enter_context(tc.tile_pool(name="y2", bufs=2))
    unT_pool = ctx.enter_context(tc.tile_pool(name="unT", bufs=2))
    plane_pool = ctx.enter_context(tc.tile_pool(name="plane", bufs=2))
    scr_pool = ctx.enter_context(tc.tile_pool(name="scr", bufs=1))
    outb_pool = ctx.enter_context(tc.tile_pool(name="outb", bufs=2))

    ps_l1 = ctx.enter_context(tc.tile_pool(name="ps_l1", bufs=2, space="PSUM"))
    ps_y2 = ctx.enter_context(tc.tile_pool(name="ps_y2", bufs=1, space="PSUM"))
    # tp halves, the 4th L2 group, natp and t2p all rotate through one
    # single-bank ring (their uses are sequential within a span)
    ps_mis = ctx.enter_context(tc.tile_pool(name="ps_mis", bufs=1, space="PSUM"))

    # constants: packed blobs (one DMA per dtype class)
    blobF = const.tile([128, 384], F32R)
    blobB = const.tile([128, 288], BF16)
    blobS = const.tile([128, 4], FP32)
    nc.sync.dma_start(blobF[:], bl1_ap[:])
    identF = blobF[:, 0:128]
    w1pads = (blobF[:, 128:256], blobF[:, 256:384])
    identB = blobB[:, 0:128]
    w2rep = blobB[:, 128:256]
    w3stk = blobB[:, 256:288]
    b1c = blobS[:, 0:1]
    b2rep = blobS[:, 1:2]
    b3c = blobS[:, 2:4]

    V_ = nc.vector
    G_ = nc.gpsimd

    planes = {}

    def group_tiles(gi):
        if gi not in planes:
            planes[gi] = {
                nm: plane_pool.tile([128, GRP * TR * (2 if nm == "uxy" else 1)],
                                    FP32, tag=nm, name=nm)
                for nm in ("relx", "rely", "hvx", "hvy", "uxy")}
        return planes[gi]

    def emit_load(span, split=False):
        obs_nat = nat_pool.tile([128, TR * NF], F32R, tag="obs_nat")
        base = span * V
        src = obs_ap[base:base + V, :].rearrange("(p t) f -> p (t f)", p=128)
        if split:
            # parallel queues at startup: halves on DVE/ACT queues while SP
            # carries the const blobs
            nc.scalar.dma_start(obs_nat[:, 0:512], src[:, 0:512])
            nc.sync.dma_start(obs_nat[:, 512:1024], src[:, 512:1024])
        else:
            nc.sync.dma_start(obs_nat[:], src)
        return obs_nat

    def emit_t1(span, obs_nat):
        gi, sl = span // GRP, span % GRP
        t = group_tiles(gi)
        # barrier input extraction (GpSimd; SBUF->SBUF)
        ob3 = obs_nat[:].rearrange("p (t f) -> p t f", f=NF)
        pl_sl = slice(sl * TR, (sl + 1) * TR)
        G_.tensor_copy(t["relx"][:, pl_sl], ob3[:, :, 6])
        G_.tensor_copy(t["rely"][:, pl_sl], ob3[:, :, 7])
        G_.tensor_copy(t["hvx"][:, pl_sl], ob3[:, :, 8])
        G_.tensor_copy(t["hvy"][:, pl_sl], ob3[:, :, 9])
        # T1: PE transpose natural -> packed obsT (rows 16*(t%8)+f)
        obsT = obsT_pool.tile([128, 1024], F32R, tag="obsT")
        for half in range(2):
            tp = ps_mis.tile([128, 512], F32R, tag="mis", name="tp")
            for ci in range(4):
                c = half * 4 + ci
                nc.tensor.matmul(
                    tp[:, ci * 128:(ci + 1) * 128],
                    obs_nat[:, c * 128:(c + 1) * 128],
                    identF, is_transpose=True)
            V_.tensor_copy(obsT[:, half * 512:(half + 1) * 512], tp[:])
        return obsT

    def emit_barrier(g, tg, nchunks=1):
        P_ = G_
        # ======== barrier math (natural layout, fp32, per group) ========
        # nchunks>1 splits the group column-wise so early chunks' chains can
        # run during later spans' MLP (used for the last group to cut tail).
        Wg = GRP * TR
        Wc = Wg // nchunks
        outb = outb_pool.tile([128, GRP * 2 * TR], FP32, tag="outb")
        for c in range(nchunks):
            ps = slice(c * Wc, (c + 1) * Wc)
            us = slice(c * 2 * Wc, (c + 1) * 2 * Wc)
            relx, rely = tg["relx"][:, ps], tg["rely"][:, ps]
            hvx, hvy = tg["hvx"][:, ps], tg["hvy"][:, ps]
            xv = tg["uxy"][:, us].rearrange("p (w c) -> p w c", c=2)
            ux, uy = xv[:, :, 0], xv[:, :, 1]
            ov = outb[:, us].rearrange("p (w c) -> p w c", c=2)

            a = scr_pool.tile([128, Wc], FP32, tag="a", name="a")
            b_ = scr_pool.tile([128, Wc], FP32, tag="b", name="b_")
            rnsq = scr_pool.tile([128, Wc], FP32, tag="rnsq", name="rnsq")
            rdm = scr_pool.tile([128, Wc], FP32, tag="rdm", name="rdm")
            sig = scr_pool.tile([128, Wc], FP32, tag="sig", name="sig")
            yv = scr_pool.tile([128, Wc], FP32, tag="yv", name="yv")
            k = scr_pool.tile([128, Wc], FP32, tag="k", name="k")
            w = scr_pool.tile([128, Wc], FP32, tag="w", name="w")

            # u_nom += b3 (in place; feeds the dot product and the output)
            V_.tensor_scalar(ux, ux, b3c[:, 0:1], None, ALU.add)
            V_.tensor_scalar(uy, uy, b3c[:, 1:2], None, ALU.add)
            # rnsq, rdm (GpSimd)
            P_.tensor_mul(a[:], relx, relx)
            P_.tensor_mul(b_[:], rely, rely)
            P_.tensor_add(rnsq[:], a[:], b_[:])
            P_.tensor_mul(a[:], hvx, relx)
            P_.tensor_mul(b_[:], hvy, rely)
            P_.tensor_add(rdm[:], a[:], b_[:])      # rel_dot_mu / 2
            # sigma = sqrt(x), x = 4*var*rnsq + eps_sig: magic rsqrt + 1 NR
            V_.tensor_scalar(sig[:], rnsq[:], 4.0 * SIG_MAX_VAR, EPS_SIG,
                             ALU.mult, ALU.add)
            V_.tensor_copy(a[:], sig[:].bitcast(I32))          # f = float(i)
            V_.tensor_scalar(a[:], a[:], -0.5, 1597463007.0, ALU.mult, ALU.add)
            V_.tensor_copy(yv[:].bitcast(I32), a[:])           # y0 bits
            P_.tensor_mul(a[:], yv[:], yv[:])
            P_.tensor_mul(a[:], a[:], sig[:])
            V_.tensor_scalar(a[:], a[:], -0.5, 1.5, ALU.mult, ALU.add)
            P_.tensor_mul(yv[:], yv[:], a[:])
            V_.tensor_mul(sig[:], sig[:], yv[:])               # sqrt = x*rsqrt
            # Precompute (rel-only, runs during MLP): P = rdm - rnsq
            # + CV/2*sigma + S^2 ; r = 1/(2*rnsq + eps/2). Post-u_nom chain
            # is then just: k = P - rel.u ; coef = max(k,0)*r ;
            # out = u + 2*coef*rel.
            V_.tensor_sub(rdm[:], rdm[:], rnsq[:])
            V_.scalar_tensor_tensor(rdm[:], sig[:], 0.5 * CVAR_COEFF, rdm[:],
                                    ALU.mult, ALU.add)
            V_.tensor_scalar(rdm[:], rdm[:], SAFE_DIST ** 2, None, ALU.add)
            V_.tensor_scalar(w[:], rnsq[:], 2.0, 0.5 * EPS_DIV,
                             ALU.mult, ALU.add)
            V_.reciprocal(w[:], w[:])
            # ---- post-u_nom chain ----
            P_.tensor_mul(a[:], relx, ux)
            P_.tensor_mul(b_[:], rely, uy)
            V_.tensor_add(a[:], a[:], b_[:])
            V_.tensor_sub(k[:], rdm[:], a[:])
            V_.tensor_scalar(k[:], k[:], 0.0, None, ALU.max)
            V_.tensor_mul(k[:], k[:], w[:])                    # coef
            # u_safe = u + 2*coef*rel
            P_.tensor_mul(a[:], k[:], relx)
            V_.scalar_tensor_tensor(ov[:, :, 0], a[:], 2.0, ux,
                                    ALU.mult, ALU.add)
            P_.tensor_mul(b_[:], k[:], rely)
            V_.scalar_tensor_tensor(ov[:, :, 1], b_[:], 2.0, uy,
                                    ALU.mult, ALU.add)

            # store run-major for the spans this chunk fully covers
            spans_per_chunk = GRP // nchunks
            for s3 in range(spans_per_chunk):
                s2 = c * spans_per_chunk + s3
                base = (g * GRP + s2) * V
                dst = out_ap[base:base + V, :].rearrange(
                    "(p t) c -> p (t c)", p=128)
                nc.sync.dma_start(dst, outb[:, s2 * 2 * TR:(s2 + 1) * 2 * TR])

    obs_nxt = emit_load(0, split=True)
    nc.sync.dma_start(blobS[:], bl3_ap[:])
    nc.sync.dma_start(blobB[:], bl2_ap[:])
    obsT_nxt = emit_t1(0, obs_nxt)

    for span in range(nspan):
        g, sl = span // GRP, span % GRP
        obsT = obsT_nxt
        tg = group_tiles(g)
        uxy = tg["uxy"]

        # layouts: obsT col = c*128 + p; row = 16*(t%8) + f; c = t//8.
        # y1sT col = (t8*2 + h)*512 + c4*128 + p   (h: half, c4 = c%4)
        # y2sT col = hs*512 + c4*128 + p, hs = 2h+sub, groups t8 = 4*sub+j
        y1sT = y1_pool.tile([128, 8192], BF16, tag="y1sT")
        y2sT = y2_pool.tile([128, 2048], BF16, tag="y2sT")

        # ---- L1: 2 concurrent row-tiles per [128,1024] psum tile ----
        for h in range(2):
            hs_cols = slice(h * 512, (h + 1) * 512)
            for par in range(2):
                for qh in range(2):
                    l1 = ps_l1.tile([128, 1024], FP32, tag="l1")
                    for qq in range(2):
                        q = 2 * qh + qq
                        nc.tensor.matmul(
                            l1[:, qq * 512:(qq + 1) * 512],
                            w1pads[par][32 * q:32 * q + 32, :],
                            obsT[32 * q:32 * q + 32, hs_cols],
                            start=True, stop=True,
                            tile_position=(32 * q, 0))
                    t80 = 4 * qh + par
                    dst = y1sT[:].rearrange("p (t8 h2 n) -> p t8 h2 n",
                                            t8=8, h2=2)[:, t80:t80 + 3:2, h]
                    srcv = l1[:].rearrange("p (q n) -> p q n", q=2)
                    nc.scalar.activation(dst, srcv, SILU,
                                         bias=b1c[:, 0:1], scale=1.0)

        # ---- pipeline next span's load + extracts + T1 ----
        if span + 1 < nspan:
            obs_nxt = emit_load(span + 1)
            obsT_nxt = emit_t1(span + 1, obs_nxt)

        # ---- L2: col-tiled K=128; hs 0-2 share one act, hs 3 via ring ----
        y2big = ps_y2.tile([128, 1536], FP32, tag="y2p")
        for hs in range(3):
            h, sub = hs // 2, hs % 2
            for j in range(4):
                t8 = 4 * sub + j
                nc.tensor.matmul(
                    y2big[32 * j:32 * j + 32, hs * 512:(hs + 1) * 512],
                    w2rep[:, 32 * j:32 * j + 32],
                    y1sT[:, (t8 * 2 + h) * 512:(t8 * 2 + h + 1) * 512],
                    start=True, stop=True,
                    tile_position=(0, 32 * j))
        if span == nspan - 1:
            # split so L3 hs=0,1 can start during the second act
            nc.scalar.activation(y2sT[:, 0:1024], y2big[:, 0:1024], SILU,
                                 bias=b2rep[:, 0:1], scale=1.0)
            nc.scalar.activation(y2sT[:, 1024:1536], y2big[:, 1024:1536], SILU,
                                 bias=b2rep[:, 0:1], scale=1.0)
        else:
            nc.scalar.activation(y2sT[:, 0:1536], y2big[:], SILU,
                                 bias=b2rep[:, 0:1], scale=1.0)
        y2d = ps_mis.tile([128, 512], FP32, tag="mis", name="y2d")
        for j in range(4):
            t8 = 4 + j
            nc.tensor.matmul(
                y2d[32 * j:32 * j + 32, :],
                w2rep[:, 32 * j:32 * j + 32],
                y1sT[:, (t8 * 2 + 1) * 512:(t8 * 2 + 2) * 512],
                start=True, stop=True,
                tile_position=(0, 32 * j))
        nc.scalar.activation(y2sT[:, 1536:2048], y2d[:], SILU,
                             bias=b2rep[:, 0:1], scale=1.0)

        # ---- L3: col-tiled block-diag w3 -> dense natural-ish psum ----
        # natp row = 32*hs + 2*j + ch, col = c4*128 + p
        natp = ps_mis.tile([128, 512], FP32, tag="mis", name="natp")
        for hs in range(4):
            nc.tensor.matmul(
                natp[32 * hs:32 * hs + 32, :],
                w3stk[:],
                y2sT[:, hs * 512:(hs + 1) * 512],
                start=True, stop=True,
                tile_position=(0, 32 * hs))
        unTt = unT_pool.tile([128, 512], BF16, tag="unT")
        V_.tensor_copy(unTt[:], natp[:])

        # ---- T2: dense transpose-back [128,128] blocks ----
        t2p = ps_mis.tile([128, 512], BF16, tag="mis", name="t2p")
        for c4 in range(4):
            nc.tensor.matmul(
                t2p[:, c4 * 128:(c4 + 1) * 128],
                unTt[:, c4 * 128:(c4 + 1) * 128],
                identB[:], is_transpose=True)
        # t2p col = c4*128 + 64h+32sub+(2j+ch) ; sample t = 32h+8c4+4sub+j
        # uxy col = sl*128 + 2t+ch = sl*128 + 64h+16c4+8sub+(2j+ch)
        inv = t2p[:].rearrange("p (c4 h sub jc) -> p h c4 sub jc",
                               c4=4, h=2, sub=2)[:, :, :, :, 0:8]
        outv = uxy[:, sl * 128:(sl + 1) * 128].rearrange(
            "p (h c4 sub jc) -> p h c4 sub jc", h=2, c4=4, sub=2)
        V_.tensor_copy(outv, inv)

        if sl == GRP - 1:
            emit_barrier(g, tg, nchunks=GRP if g == ngrp - 1 else 1)
            del planes[g]


def prep_consts(w1, b1, w2, b2, w3, b3):
    w1pad0 = np.zeros((128, 128), np.float32)
    w1pad1 = np.zeros((128, 128), np.float32)
    w2rep = np.zeros((128, 128), BF)
    w3stack = np.zeros((128, 32), BF)
    for q in range(4):
        w1pad0[32 * q:32 * q + 16, :] = w1.T          # even t8 groups
        w1pad1[32 * q + 16:32 * q + 32, :] = w1.T     # odd t8 groups
    for j in range(4):
        w2rep[:, 32 * j:32 * j + 32] = w2.T.astype(BF)
        w3stack[32 * j:32 * j + 32, 2 * j:2 * j + 2] = w3.T.astype(BF)
    b3c = np.empty((128, 2), np.float32)
    b3c[:, 0] = b3[0]
    b3c[:, 1] = b3[1]
    blobF = np.concatenate([np.eye(128, dtype=np.float32), w1pad0, w1pad1],
                           axis=1)
    blobB = np.concatenate([np.eye(128, dtype=BF), w2rep, w3stack], axis=1)
    blobS = np.concatenate([
        np.asarray(b1, np.float32).reshape(128, 1),
        np.tile(np.asarray(b2, np.float32), 4).reshape(128, 1),
        b3c], axis=1)
    return dict(blobF=np.ascontiguousarray(blobF),
                blobB=np.ascontiguousarray(blobB),
                blobS=np.ascontiguousarray(blobS))


def kernel(obs, w1, b1, w2, b2, w3, b3):
    obs = np.asarray(obs, np.float32)
    consts = prep_consts(np.asarray(w1, np.float32), np.asarray(b1, np.float32),
                         np.asarray(w2, np.float32), np.asarray(b2, np.float32),
                         np.asarray(w3, np.float32), np.asarray(b3, np.float32))
    if "nc" not in _cached:
        _cached["nc"] = build()
    nc = _cached["nc"]
    in_maps = []
    for c in range(N_CORES):
        m = {"obs": np.ascontiguousarray(obs[c * S:(c + 1) * S])}
        m.update(consts)
        in_maps.append(m)
    res = run_bass_kernel_spmd(nc, in_maps, list(range(N_CORES)))
    out = np.empty((B, NC), np.float32)
    for c in range(N_CORES):
        out[c * S:(c + 1) * S] = res.results[c]["out"]
    return out
